# revision 68
# baseline (speedup 1.0000x reference)
"""Trainium2 Bass kernel for ErnieLayout self-attention (B=4,S=1024,H=768,NH=12,HD=64).

Sharding: 8 cores = 4 batches x 2 head-groups (6 heads each).

Key restructuring vs the straightforward version:
- The attention mask is known host-side, so the k dimension is COMPACTED to
  the unmasked key positions (~50%), padded to a multiple of 64 (KPAD).
  Masked keys contribute exp(-1e10)=0 to softmax, identical to dropping them.
- hs is shipped pre-transposed ([d, s]) so no PE transposes are needed for
  the projections; K/V project only the compacted key rows.
- rel_pos + rel_2d_pos are pre-added, exponentiated, compacted and shipped
  TRANSPOSED ([k, q] layout) as E = exp(rel1+rel2); since
  softmax(s + r) ~ exp(s)*exp(r), the bias merge becomes a bf16 DVE multiply
  (2x mode) instead of PE transposes + PSUM accumulation.
- Scores are computed in [k, q] layout with K=64 contraction via
  base_partition slicing (heads of a pair live on partitions 0-63/64-127).
- PV uses the [V|ones] trick: the 65th output row is the softmax
  denominator. Un-normalized [65, 512] tiles are DMA'd out; the host
  divides and transposes (cheap, removes on-device transposes/reciprocals).
- All inputs are shipped pre-rearranged to the on-chip [partition, chunk,
  free] layout so every DMA is 128 large contiguous per-partition
  descriptors (per-descriptor overhead otherwise halves DMA throughput).
- Only the sync queue is hardware-DGE (other engines issue software DGE at
  ~1/5 the bandwidth), so every sizable tensor streams on sync in
  arrival-priority order.
- Engines execute their instruction streams in order, so the emission
  order is arranged to match DMA arrival: per-head-pair weight slices,
  per-s-half hs slices, and each pair's PV lagging one pair behind its
  scores so the in-order PE stream never blocks on a late E tile.
- Engine balance: ACT does only the exps; projection bias+cast and the
  rel-multiplies and output casts run on DVE; gpsimd only carries the tiny
  bias broadcast (its tensor ops measured ~3x slower than DVE).
"""
import os
import numpy as np
import ml_dtypes

from concourse import bacc, mybir, tile
from concourse.bass_utils import run_bass_kernel_spmd
import concourse.bass as bass

B, S, H = 4, 1024, 768
NH, HD = 12, 64
N_CORES = 8
HPC = 6            # heads per core
NHP = HPC // 2     # head pairs per core
COLS = HPC * HD    # 384 projection output columns per core
KC = H // 128      # 6 contraction chunks for projections
QH = 2             # q halves of 512
bf16 = mybir.dt.bfloat16
f32 = mybir.dt.float32
AF = mybir.ActivationFunctionType
BF16_NP = ml_dtypes.bfloat16

_compiled = {}
last_result = None  # BassKernelResults of the most recent run (for test harness)


def _build(kpad):
    n_full = kpad // 128       # full 128-row k chunks
    rem = kpad % 128           # 0 or 64 (kpad is a multiple of 64)
    kch = n_full + (1 if rem else 0)
    # (offset, size) of each compacted-k chunk
    chunks = [(i * 128, 128) for i in range(n_full)]
    if rem:
        chunks.append((n_full * 128, rem))

    nc = bacc.Bacc("TRN2", target_bir_lowering=False, debug=False,
                   num_devices=N_CORES)
    hsT = nc.dram_tensor("hsT", [128, QH, KC, 512], bf16,
                         kind="ExternalInput").ap()
    hkT = nc.dram_tensor("hkT", [128, KC, kpad], bf16, kind="ExternalInput").ap()
    wq = nc.dram_tensor("wq", [128, NHP, KC, 128], bf16,
                        kind="ExternalInput").ap()
    wk = nc.dram_tensor("wk", [128, NHP, KC, 128], bf16,
                        kind="ExternalInput").ap()
    wv = nc.dram_tensor("wv", [128, KC, COLS], bf16, kind="ExternalInput").ap()
    bq = nc.dram_tensor("bq", [COLS], f32, kind="ExternalInput").ap()
    bk = nc.dram_tensor("bk", [COLS], f32, kind="ExternalInput").ap()
    bv = nc.dram_tensor("bv", [COLS], f32, kind="ExternalInput").ap()
    ehA = nc.dram_tensor("ehA", [HPC, 128, n_full, S], bf16,
                         kind="ExternalInput").ap()
    ehB = (nc.dram_tensor("ehB", [HPC, rem, S], bf16,
                          kind="ExternalInput").ap() if rem else None)
    outT = nc.dram_tensor("outT", [HPC, HD + 1, S], bf16,
                          kind="ExternalOutput").ap()

    with tile.TileContext(nc) as tc:
        with tc.tile_pool(name="const", bufs=1) as const, \
             tc.tile_pool(name="hst", bufs=1) as hst_pool, \
             tc.tile_pool(name="w", bufs=1) as w_pool, \
             tc.tile_pool(name="qk", bufs=1) as qk_pool, \
             tc.tile_pool(name="v", bufs=1) as v_pool, \
             tc.tile_pool(name="e", bufs=HPC) as e_pool, \
             tc.tile_pool(name="et0", bufs=3) as et0_pool, \
             tc.tile_pool(name="et", bufs=6) as et_pool, \
             tc.tile_pool(name="ob", bufs=4) as ob_pool:

            wq_sb = w_pool.tile([128, NHP, KC, 128], bf16)
            wk_sb = w_pool.tile([128, NHP, KC, 128], bf16)
            wv_sb = w_pool.tile([128, KC, COLS], bf16)
            hsT_sb = hst_pool.tile([128, QH, KC, 512], bf16)
            hkT_sb = hst_pool.tile([128, KC, kpad], bf16)

            # tiny bias loads on gpsimd (SWDGE): each sync-queue DMA issue
            # costs ~1us of issue+sem latency, which would delay the
            # critical hsT0/wq0 arrivals behind them
            bq_sb = const.tile([128, NHP], f32)
            nc.gpsimd.dma_start(out=bq_sb, in_=bq.rearrange("(c p) -> p c", p=128))
            bk_sb = const.tile([128, NHP], f32)
            nc.gpsimd.dma_start(out=bk_sb, in_=bk.rearrange("(c p) -> p c", p=128))
            bv_bc = bass.AP(tensor=bv.tensor, offset=bv.offset,
                            ap=[[0, 128]] + list(bv.ap))
            bv_sb = const.tile([128, COLS], f32)
            nc.gpsimd.dma_start(out=bv_sb, in_=bv_bc)

            nc.sync.dma_start(out=wq_sb[:, 0], in_=wq[:, 0])
            nc.sync.dma_start(out=wk_sb[:, 0], in_=wk[:, 0])
            nc.sync.dma_start(out=hsT_sb[:, 0], in_=hsT[:, 0])
            nc.sync.dma_start(out=hkT_sb, in_=hkT)
            nc.sync.dma_start(out=wv_sb, in_=wv)
            nc.sync.dma_start(out=hsT_sb[:, 1], in_=hsT[:, 1])

            def emit_e_load(h):
                e_t = e_pool.tile([128, kch, S], bf16, tag="e")
                nc.sync.dma_start(out=e_t[:, 0:n_full, :], in_=ehA[h])
                if rem:
                    nc.sync.dma_start(out=e_t[0:rem, n_full, :], in_=ehB[h])
                return e_t

            es = [None] * HPC
            es[0] = emit_e_load(0)
            es[1] = emit_e_load(1)
            nc.sync.dma_start(out=wq_sb[:, 1], in_=wq[:, 1])
            nc.sync.dma_start(out=wk_sb[:, 1], in_=wk[:, 1])
            es[2] = emit_e_load(2)
            nc.sync.dma_start(out=wq_sb[:, 2], in_=wq[:, 2])
            nc.sync.dma_start(out=wk_sb[:, 2], in_=wk[:, 2])
            es[3] = emit_e_load(3)
            es[4] = emit_e_load(4)
            es[5] = emit_e_load(5)

            # exp table load warm-up: tiny activation so ACT_TABLE_LOAD runs
            # during the startup DMA window
            dummy = const.tile([128, 1], f32)
            nc.vector.memset(dummy, 0.0)
            dummy2 = const.tile([128, 1], bf16)
            nc.scalar.activation(out=dummy2, in_=dummy, func=AF.Exp)

            _psum_cms = [tc.tile_pool(name="pp", bufs=2, space="PSUM"),
                         tc.tile_pool(name="sc2", bufs=2, space="PSUM"),
                         tc.tile_pool(name="pv", bufs=2, space="PSUM")]
            pp_psum, sc_psum, pv_psum = (cm.__enter__() for cm in _psum_cms)

            # HAM warmup: dependency-free matmuls on an unwritten tile run
            # during the startup DMA window, flipping the PE clock gate to
            # 2.4GHz before the real projections arrive.
            # HAM warmup sized to bridge until hsT0/wq0 land (~13us): a
            # shorter warmup lets the PE idle >3.4us and the first Q/K
            # projection chains then run at half clock (427 ns/MM measured)
            garbage = const.tile([128, 384], bf16)
            nc.vector.memset(garbage, 0.0)
            warm = sc_psum.tile([128, 1024], f32, tag="sc")
            for _ in range(36):
                nc.tensor.matmul(warm[:, 0:256], garbage[:, 0:128],
                                 garbage[:, 128:384], start=True, stop=True)

            # qT/kT: [d(2 heads stacked on partitions), s|k] per head pair.
            # q pre-scaled by 1/8 (folded into Wq/bq host-side).
            qT = qk_pool.tile([128, NHP, S], bf16)
            kT = qk_pool.tile([128, NHP, kpad], bf16)
            v_sb = v_pool.tile([128, kch, HPC, HD + 1], bf16)
            nc.gpsimd.memset(v_sb[:, :, :, HD], 1.0)

            def emit_proj_q(hp, sh):
                # bias-add + cast on DVE so the scalar engine is dedicated
                # to exps
                ssl = slice(sh * 512, (sh + 1) * 512)
                psq = pp_psum.tile([128, 512], f32, tag="pp")
                for k in range(KC):
                    nc.tensor.matmul(psq, wq_sb[:, hp, k, :],
                                     hsT_sb[:, sh, k, :],
                                     start=(k == 0), stop=(k == KC - 1))
                nc.vector.tensor_scalar_add(qT[:, hp, ssl], psq,
                                            bq_sb[:, hp:hp + 1])

            def emit_proj_k(hp):
                for k0 in range(0, kpad, 512):
                    kw = min(512, kpad - k0)
                    ksl = slice(k0, k0 + kw)
                    psk = pp_psum.tile([128, 512], f32, tag="pp")
                    for k in range(KC):
                        nc.tensor.matmul(psk[:, 0:kw], wk_sb[:, hp, k, :],
                                         hkT_sb[:, k, ksl],
                                         start=(k == 0), stop=(k == KC - 1))
                    nc.vector.tensor_scalar_add(kT[:, hp, ksl], psk[:, 0:kw],
                                                bk_sb[:, hp:hp + 1])

            def emit_proj_v():
                for ci, (off, sz) in enumerate(chunks):
                    psv_full = pp_psum.tile([128, 512], f32, tag="pp")
                    psv = psv_full[0:sz, 0:COLS]
                    for k in range(KC):
                        nc.tensor.matmul(psv, hkT_sb[:, k, off:off + sz],
                                         wv_sb[:, k, :],
                                         start=(k == 0), stop=(k == KC - 1))
                    nc.vector.tensor_add(
                        v_sb[0:sz, ci, :, 0:HD],
                        psv.rearrange("p (h d) -> p h d", h=HPC),
                        bv_sb[0:sz].rearrange("p (h d) -> p h d", h=HPC))

            # kc chunk groups: pairs of 2 (one sc2 tile per head), plus a
            # trailing chunk shared between the two heads of the pair.
            kc_pairs = [(g, g + 1) for g in range(0, kch - 1, 2)]
            kc_single = kch - 1 if kch % 2 else None

            def emit_scores(hp, qh):
                """scores + exp + rel-multiply for both heads of pair hp.

                The rel-multiply is split per kc-chunk-group so the PV
                accumulation can start as soon as its first chunks are
                ready instead of waiting for the whole unit's numerators.
                """
                e_ts = (es[2 * hp], es[2 * hp + 1])
                qsl = slice(qh * 512, (qh + 1) * 512)
                et0 = et0_pool.tile([128, 2, kch, 512], bf16, tag="et0")
                ets = [et_pool.tile([128, kch, 512], bf16, tag="et",
                                    name=f"et{hi}")
                       for hi in range(2)]
                for (ka, kb) in kc_pairs:
                    for hi in range(2):
                        psl = slice(hi * 64, (hi + 1) * 64)
                        ps = sc_psum.tile([128, 1024], f32, tag="sc")
                        for j, kc_i in enumerate((ka, kb)):
                            off, sz = chunks[kc_i]
                            nc.tensor.matmul(
                                ps[0:sz, j * 512:(j + 1) * 512],
                                kT[psl, hp, off:off + sz],
                                qT[psl, hp, qsl], start=True, stop=True)
                        nc.scalar.activation(
                            out=et0[:, hi, ka:ka + 2, :],
                            in_=ps.rearrange("p (u q) -> p u q", u=2),
                            func=AF.Exp)
                        nc.vector.tensor_mul(ets[hi][:, ka:ka + 2, :],
                                             et0[:, hi, ka:ka + 2, :],
                                             e_ts[hi][:, ka:ka + 2, qsl])
                if kc_single is not None:
                    kc_i = kc_single
                    off, sz = chunks[kc_i]
                    ps = sc_psum.tile([128, 1024], f32, tag="sc")
                    for hi in range(2):
                        psl = slice(hi * 64, (hi + 1) * 64)
                        nc.tensor.matmul(
                            ps[0:sz, hi * 512:(hi + 1) * 512],
                            kT[psl, hp, off:off + sz],
                            qT[psl, hp, qsl], start=True, stop=True)
                    nc.scalar.activation(
                        out=et0[0:sz, :, kc_i, :],
                        in_=ps[0:sz].rearrange("p (u q) -> p u q", u=2),
                        func=AF.Exp)
                    for hi in range(2):
                        nc.vector.tensor_mul(ets[hi][0:sz, kc_i, :],
                                             et0[0:sz, hi, kc_i, :],
                                             e_ts[hi][0:sz, kc_i, qsl])
                return ets

            def emit_pv(state, act_cast=False):
                (hp, qh, ets) = state
                for hi in range(2):
                    h = 2 * hp + hi
                    pv = pv_psum.tile([HD + 1, 512], f32, tag="pv")
                    for kc_i, (off, sz) in enumerate(chunks):
                        nc.tensor.matmul(pv, v_sb[0:sz, kc_i, h, :],
                                         ets[hi][0:sz, kc_i, :],
                                         start=(kc_i == 0),
                                         stop=(kc_i == kch - 1))
                    ob = ob_pool.tile([HD + 1, 512], bf16, tag="ob")
                    if act_cast:
                        # tail pairs: cast on the (by then idle) scalar
                        # engine so the pv psum slots don't wait behind the
                        # final muls in the DVE stream
                        nc.scalar.copy(ob, pv)
                    else:
                        nc.vector.tensor_copy(ob, pv)
                    nc.sync.dma_start(
                        out=outT[h, :, qh * 512:(qh + 1) * 512], in_=ob)

            # ---- schedule: emission order == per-engine execution order, so
            # it is arranged to match DMA arrival; PV lags its pair's scores
            # by one pair so the in-order PE stream never blocks on a late E
            # arrival ----
            pend = []

            def pair(hp, qh, act_cast=False):
                ets = emit_scores(hp, qh)
                if pend:
                    emit_pv(pend.pop(), act_cast)
                pend.append((hp, qh, ets))

            emit_proj_q(0, 0)
            emit_proj_k(0)
            pair(0, 0)
            emit_proj_v()
            emit_proj_q(0, 1)
            pair(0, 1)
            emit_proj_q(1, 0)
            emit_proj_q(1, 1)
            emit_proj_k(1)
            pair(1, 0)
            pair(1, 1)
            emit_proj_q(2, 0)
            emit_proj_q(2, 1)
            emit_proj_k(2)
            pair(2, 0)
            pair(2, 1, act_cast=True)
            emit_pv(pend.pop(), act_cast=True)

            for cm in reversed(_psum_cms):
                cm.__exit__(None, None, None)

    nc.compile()
    return nc


def _get_compiled(kpad):
    if kpad not in _compiled:
        _compiled[kpad] = _build(kpad)
    return _compiled[kpad]


def kernel(hidden_states, Wq, bq, Wk, bk, Wv, bv, rel_pos, rel_2d_pos,
           attention_mask, _trace=False):
    global last_result

    hidden_states = np.asarray(hidden_states, np.float32)
    Wq, Wk, Wv = (np.asarray(w, np.float32) for w in (Wq, Wk, Wv))
    bq, bk, bv = (np.asarray(x, np.float32) for x in (bq, bk, bv))
    rel_pos = np.asarray(rel_pos, np.float32)
    rel_2d_pos = np.asarray(rel_2d_pos, np.float32)
    attention_mask = np.asarray(attention_mask, np.int32)

    # compact k to unmasked key positions (masked keys get probability 0)
    keeps = [np.where(attention_mask[b, 0, 0] == 0)[0] for b in range(B)]
    max_kc = max(len(k) for k in keeps)
    kpad = max(128, -(-max_kc // 64) * 64)
    n_full, rem = kpad // 128, kpad % 128
    nc = _get_compiled(kpad)

    in_maps = []
    for c in range(N_CORES):
        b, hg = divmod(c, 2)
        cs = slice(hg * COLS, (hg + 1) * COLS)
        h0 = hg * HPC
        keep = keeps[b]
        k_c = len(keep)
        hkT = np.zeros((H, kpad), BF16_NP)
        hkT[:, :k_c] = hidden_states[b][keep].T
        # E = exp(rel1+rel2) compacted along k and transposed to [k, q];
        # zero at padding -> those keys get weight exactly 0.
        r = (rel_pos[b, h0:h0 + HPC][:, :, keep]
             + rel_2d_pos[b, h0:h0 + HPC][:, :, keep])
        eh = np.zeros((HPC, kpad, S), BF16_NP)
        eh[:, :k_c, :] = np.exp(r).transpose(0, 2, 1)

        def onchip(a, nchunk):
            # [c*128+p, n] -> [p, c, n] (pre-applied DMA rearrange)
            return np.ascontiguousarray(
                a.reshape(nchunk, 128, a.shape[-1]).transpose(1, 0, 2))

        def onchip_hp(a):
            # [c*128+p, hp*128+m] -> [p, hp, c, m]
            return np.ascontiguousarray(
                a.reshape(KC, 128, NHP, 128).transpose(1, 2, 0, 3))

        im = {
            # [p, sh, c, m] layout (sh-major so each half unblocks early)
            "hsT": np.ascontiguousarray(
                hidden_states[b].T.astype(BF16_NP)
                .reshape(KC, 128, QH, 512).transpose(1, 2, 0, 3)),
            "hkT": onchip(hkT, KC),
            "wq": onchip_hp((Wq[:, cs] * np.float32(0.125)).astype(BF16_NP)),
            "wk": onchip_hp(Wk[:, cs].astype(BF16_NP)),
            "wv": onchip(Wv[:, cs].astype(BF16_NP), KC),
            "bq": np.ascontiguousarray(bq[cs]) * np.float32(0.125),
            "bk": np.ascontiguousarray(bk[cs]),
            "bv": np.ascontiguousarray(bv[cs]),
            # [h, p, c, q] for the full 128-chunks; remainder separate
            "ehA": np.ascontiguousarray(
                eh[:, :n_full * 128, :].reshape(HPC, n_full, 128, S)
                .transpose(0, 2, 1, 3)),
        }
        if rem:
            im["ehB"] = np.ascontiguousarray(eh[:, n_full * 128:, :])
        in_maps.append(im)

    kwargs = {}
    if _trace or os.environ.get("KERNEL_TRACE"):
        kwargs["trace"] = True
    last_result = run_bass_kernel_spmd(nc, in_maps, list(range(N_CORES)), **kwargs)

    result = np.empty((B, S, H), np.float32)
    for c in range(N_CORES):
        b, hg = divmod(c, 2)
        ot = np.asarray(last_result.results[c]["outT"], np.float32)
        ctx = ot[:, 0:HD, :] / ot[:, HD:HD + 1, :]       # [HPC, HD, S]
        result[b, :, hg * COLS:(hg + 1) * COLS] = (
            ctx.transpose(2, 0, 1).reshape(S, COLS))
    return result


# revision 69
# speedup vs baseline: 1.0713x; 1.0713x over previous
"""Trainium2 Bass kernel for ErnieLayout self-attention (B=4,S=1024,H=768,NH=12,HD=64).

Sharding: 8 cores = 4 batches x 2 head-groups (6 heads each).

Key restructuring vs the straightforward version:
- The attention mask is known host-side, so the k dimension is COMPACTED to
  the unmasked key positions (~50%), padded to a multiple of 64 (KPAD).
  Masked keys contribute exp(-1e10)=0 to softmax, identical to dropping them.
- hs is shipped pre-transposed ([d, s]) so no PE transposes are needed for
  the projections; K/V project only the compacted key rows.
- rel_pos + rel_2d_pos are pre-added, exponentiated, compacted and shipped
  TRANSPOSED ([k, q] layout) as E = exp(rel1+rel2); since
  softmax(s + r) ~ exp(s)*exp(r), the bias merge becomes a bf16 DVE multiply
  (2x mode) instead of PE transposes + PSUM accumulation.
- Scores are computed in [k, q] layout with K=64 contraction via
  base_partition slicing (heads of a pair live on partitions 0-63/64-127).
- PV uses the [V|ones] trick: the 65th output row is the softmax
  denominator. Un-normalized [65, 512] tiles are DMA'd out; the host
  divides and transposes (cheap, removes on-device transposes/reciprocals).
- All inputs are shipped pre-rearranged to the on-chip [partition, chunk,
  free] layout so every DMA is 128 large contiguous per-partition
  descriptors (per-descriptor overhead otherwise halves DMA throughput).
- Only the sync queue is hardware-DGE (other engines issue software DGE at
  ~1/5 the bandwidth), so every sizable tensor streams on sync in
  arrival-priority order.
- Engines execute their instruction streams in order, so the emission
  order is arranged to match DMA arrival: per-head-pair weight slices,
  per-s-half hs slices, and each pair's PV lagging one pair behind its
  scores so the in-order PE stream never blocks on a late E tile.
- Engine balance: ACT does only the exps; projection bias+cast and the
  rel-multiplies and output casts run on DVE; gpsimd only carries the tiny
  bias broadcast (its tensor ops measured ~3x slower than DVE).
"""
import os
import numpy as np
import ml_dtypes

from concourse import bacc, mybir, tile
from concourse.bass_utils import run_bass_kernel_spmd
import concourse.bass as bass

B, S, H = 4, 1024, 768
NH, HD = 12, 64
N_CORES = 8
HPC = 6            # heads per core
NHP = HPC // 2     # head pairs per core
COLS = HPC * HD    # 384 projection output columns per core
KC = H // 128      # 6 contraction chunks for projections
QH = 2             # q halves of 512
bf16 = mybir.dt.bfloat16
f32 = mybir.dt.float32
AF = mybir.ActivationFunctionType
BF16_NP = ml_dtypes.bfloat16

_compiled = {}
last_result = None  # BassKernelResults of the most recent run (for test harness)


def _build(kpad):
    n_full = kpad // 128       # full 128-row k chunks
    rem = kpad % 128           # 0 or 64 (kpad is a multiple of 64)
    kch = n_full + (1 if rem else 0)
    # (offset, size) of each compacted-k chunk
    chunks = [(i * 128, 128) for i in range(n_full)]
    if rem:
        chunks.append((n_full * 128, rem))

    nc = bacc.Bacc("TRN2", target_bir_lowering=False, debug=False,
                   num_devices=N_CORES)
    hsT = nc.dram_tensor("hsT", [128, QH, KC, 512], bf16,
                         kind="ExternalInput").ap()
    hkT = nc.dram_tensor("hkT", [128, KC, kpad], bf16, kind="ExternalInput").ap()
    wq = nc.dram_tensor("wq", [128, NHP, KC, 128], bf16,
                        kind="ExternalInput").ap()
    wk = nc.dram_tensor("wk", [128, NHP, KC, 128], bf16,
                        kind="ExternalInput").ap()
    wv = nc.dram_tensor("wv", [128, KC, COLS], bf16, kind="ExternalInput").ap()
    bq = nc.dram_tensor("bq", [COLS], f32, kind="ExternalInput").ap()
    bk = nc.dram_tensor("bk", [COLS], f32, kind="ExternalInput").ap()
    bv = nc.dram_tensor("bv", [COLS], f32, kind="ExternalInput").ap()
    ehA = nc.dram_tensor("ehA", [HPC, 128, n_full, S], bf16,
                         kind="ExternalInput").ap()
    ehB = (nc.dram_tensor("ehB", [HPC, rem, S], bf16,
                          kind="ExternalInput").ap() if rem else None)
    outT = nc.dram_tensor("outT", [HPC, HD + 1, S], bf16,
                          kind="ExternalOutput").ap()

    with tile.TileContext(nc) as tc:
        with tc.tile_pool(name="const", bufs=1) as const, \
             tc.tile_pool(name="hst", bufs=1) as hst_pool, \
             tc.tile_pool(name="w", bufs=1) as w_pool, \
             tc.tile_pool(name="qk", bufs=1) as qk_pool, \
             tc.tile_pool(name="v", bufs=1) as v_pool, \
             tc.tile_pool(name="e", bufs=HPC) as e_pool, \
             tc.tile_pool(name="et0", bufs=3) as et0_pool, \
             tc.tile_pool(name="et", bufs=6) as et_pool, \
             tc.tile_pool(name="ob", bufs=4) as ob_pool:

            wq_sb = w_pool.tile([128, NHP, KC, 128], bf16)
            wk_sb = w_pool.tile([128, NHP, KC, 128], bf16)
            wv_sb = w_pool.tile([128, KC, COLS], bf16)
            hsT_sb = hst_pool.tile([128, QH, KC, 512], bf16)
            hkT_sb = hst_pool.tile([128, KC, kpad], bf16)

            # tiny bias loads on gpsimd (SWDGE): each sync-queue DMA issue
            # costs ~1us of issue+sem latency, which would delay the
            # critical hsT0/wq0 arrivals behind them
            bq_sb = const.tile([128, NHP], f32)
            nc.gpsimd.dma_start(out=bq_sb, in_=bq.rearrange("(c p) -> p c", p=128))
            bk_sb = const.tile([128, NHP], f32)
            nc.gpsimd.dma_start(out=bk_sb, in_=bk.rearrange("(c p) -> p c", p=128))
            bv_bc = bass.AP(tensor=bv.tensor, offset=bv.offset,
                            ap=[[0, 128]] + list(bv.ap))
            bv_sb = const.tile([128, COLS], f32)
            nc.gpsimd.dma_start(out=bv_sb, in_=bv_bc)

            nc.sync.dma_start(out=wq_sb[:, 0], in_=wq[:, 0])
            nc.sync.dma_start(out=wk_sb[:, 0], in_=wk[:, 0])
            nc.sync.dma_start(out=hsT_sb[:, 0], in_=hsT[:, 0])
            nc.sync.dma_start(out=hkT_sb, in_=hkT)
            nc.sync.dma_start(out=wv_sb, in_=wv)
            nc.sync.dma_start(out=hsT_sb[:, 1], in_=hsT[:, 1])

            def emit_e_load(h):
                e_t = e_pool.tile([128, kch, S], bf16, tag="e")
                nc.sync.dma_start(out=e_t[:, 0:n_full, :], in_=ehA[h])
                if rem:
                    nc.sync.dma_start(out=e_t[0:rem, n_full, :], in_=ehB[h])
                return e_t

            es = [None] * HPC
            es[0] = emit_e_load(0)
            es[1] = emit_e_load(1)
            nc.sync.dma_start(out=wq_sb[:, 1], in_=wq[:, 1])
            nc.sync.dma_start(out=wk_sb[:, 1], in_=wk[:, 1])
            es[2] = emit_e_load(2)
            nc.sync.dma_start(out=wq_sb[:, 2], in_=wq[:, 2])
            nc.sync.dma_start(out=wk_sb[:, 2], in_=wk[:, 2])
            es[3] = emit_e_load(3)
            es[4] = emit_e_load(4)
            es[5] = emit_e_load(5)

            # exp table load warm-up: tiny activation so ACT_TABLE_LOAD runs
            # during the startup DMA window
            dummy = const.tile([128, 1], f32)
            nc.vector.memset(dummy, 0.0)
            dummy2 = const.tile([128, 1], bf16)
            nc.scalar.activation(out=dummy2, in_=dummy, func=AF.Exp)

            _psum_cms = [tc.tile_pool(name="pp", bufs=2, space="PSUM"),
                         tc.tile_pool(name="sc2", bufs=2, space="PSUM"),
                         tc.tile_pool(name="pv", bufs=2, space="PSUM")]
            pp_psum, sc_psum, pv_psum = (cm.__enter__() for cm in _psum_cms)

            # HAM warmup: dependency-free matmuls on an unwritten tile run
            # during the startup DMA window, flipping the PE clock gate to
            # 2.4GHz before the real projections arrive.
            # HAM warmup sized to bridge until hsT0/wq0 land (~13us): a
            # shorter warmup lets the PE idle >3.4us and the first Q/K
            # projection chains then run at half clock (427 ns/MM measured)
            garbage = const.tile([128, 384], bf16)
            nc.vector.memset(garbage, 0.0)
            warm = sc_psum.tile([128, 1024], f32, tag="sc")
            for _ in range(36):
                nc.tensor.matmul(warm[:, 0:256], garbage[:, 0:128],
                                 garbage[:, 128:384], start=True, stop=True)

            # qT/kT: [d(2 heads stacked on partitions), s|k] per head pair.
            # q pre-scaled by 1/8 (folded into Wq/bq host-side).
            qT = qk_pool.tile([128, NHP, S], bf16)
            kT = qk_pool.tile([128, NHP, kpad], bf16)
            v_sb = v_pool.tile([128, kch, HPC, HD + 1], bf16)
            nc.gpsimd.memset(v_sb[:, :, :, HD], 1.0)

            def emit_proj_q(hp, sh):
                # bias-add + cast on DVE so the scalar engine is dedicated
                # to exps
                ssl = slice(sh * 512, (sh + 1) * 512)
                psq = pp_psum.tile([128, 512], f32, tag="pp")
                for k in range(KC):
                    nc.tensor.matmul(psq, wq_sb[:, hp, k, :],
                                     hsT_sb[:, sh, k, :],
                                     start=(k == 0), stop=(k == KC - 1))
                nc.vector.tensor_scalar_add(qT[:, hp, ssl], psq,
                                            bq_sb[:, hp:hp + 1])

            def emit_proj_k(hp):
                for k0 in range(0, kpad, 512):
                    kw = min(512, kpad - k0)
                    ksl = slice(k0, k0 + kw)
                    psk = pp_psum.tile([128, 512], f32, tag="pp")
                    for k in range(KC):
                        nc.tensor.matmul(psk[:, 0:kw], wk_sb[:, hp, k, :],
                                         hkT_sb[:, k, ksl],
                                         start=(k == 0), stop=(k == KC - 1))
                    nc.vector.tensor_scalar_add(kT[:, hp, ksl], psk[:, 0:kw],
                                                bk_sb[:, hp:hp + 1])

            def emit_proj_v():
                for ci, (off, sz) in enumerate(chunks):
                    psv_full = pp_psum.tile([128, 512], f32, tag="pp")
                    psv = psv_full[0:sz, 0:COLS]
                    for k in range(KC):
                        nc.tensor.matmul(psv, hkT_sb[:, k, off:off + sz],
                                         wv_sb[:, k, :],
                                         start=(k == 0), stop=(k == KC - 1))
                    nc.vector.tensor_add(
                        v_sb[0:sz, ci, :, 0:HD],
                        psv.rearrange("p (h d) -> p h d", h=HPC),
                        bv_sb[0:sz].rearrange("p (h d) -> p h d", h=HPC))

            # kc chunk groups: pairs of 2 (one sc2 tile per head), plus a
            # trailing chunk shared between the two heads of the pair.
            kc_pairs = [(g, g + 1) for g in range(0, kch - 1, 2)]
            kc_single = kch - 1 if kch % 2 else None

            def emit_scores(hp, qh):
                """scores + exp + rel-multiply for both heads of pair hp.

                The rel-multiply is split per kc-chunk-group so the PV
                accumulation can start as soon as its first chunks are
                ready instead of waiting for the whole unit's numerators.
                """
                e_ts = (es[2 * hp], es[2 * hp + 1])
                qsl = slice(qh * 512, (qh + 1) * 512)
                et0 = et0_pool.tile([128, 2, kch, 512], bf16, tag="et0")
                ets = [et_pool.tile([128, kch, 512], bf16, tag="et",
                                    name=f"et{hi}")
                       for hi in range(2)]
                for (ka, kb) in kc_pairs:
                    for hi in range(2):
                        psl = slice(hi * 64, (hi + 1) * 64)
                        ps = sc_psum.tile([128, 1024], f32, tag="sc")
                        for j, kc_i in enumerate((ka, kb)):
                            off, sz = chunks[kc_i]
                            nc.tensor.matmul(
                                ps[0:sz, j * 512:(j + 1) * 512],
                                kT[psl, hp, off:off + sz],
                                qT[psl, hp, qsl], start=True, stop=True)
                        nc.scalar.activation(
                            out=et0[:, hi, ka:ka + 2, :],
                            in_=ps.rearrange("p (u q) -> p u q", u=2),
                            func=AF.Exp)
                        nc.vector.tensor_mul(ets[hi][:, ka:ka + 2, :],
                                             et0[:, hi, ka:ka + 2, :],
                                             e_ts[hi][:, ka:ka + 2, qsl])
                if kc_single is not None:
                    kc_i = kc_single
                    off, sz = chunks[kc_i]
                    ps = sc_psum.tile([128, 1024], f32, tag="sc")
                    for hi in range(2):
                        psl = slice(hi * 64, (hi + 1) * 64)
                        nc.tensor.matmul(
                            ps[0:sz, hi * 512:(hi + 1) * 512],
                            kT[psl, hp, off:off + sz],
                            qT[psl, hp, qsl], start=True, stop=True)
                    nc.scalar.activation(
                        out=et0[0:sz, :, kc_i, :],
                        in_=ps[0:sz].rearrange("p (u q) -> p u q", u=2),
                        func=AF.Exp)
                    for hi in range(2):
                        nc.vector.tensor_mul(ets[hi][0:sz, kc_i, :],
                                             et0[0:sz, hi, kc_i, :],
                                             e_ts[hi][0:sz, kc_i, qsl])
                return ets

            def emit_pv(state, act_cast=False):
                (hp, qh, ets) = state
                for hi in range(2):
                    h = 2 * hp + hi
                    pv = pv_psum.tile([HD + 1, 512], f32, tag="pv")
                    for kc_i, (off, sz) in enumerate(chunks):
                        nc.tensor.matmul(pv, v_sb[0:sz, kc_i, h, :],
                                         ets[hi][0:sz, kc_i, :],
                                         start=(kc_i == 0),
                                         stop=(kc_i == kch - 1))
                    ob = ob_pool.tile([HD + 1, 512], bf16, tag="ob")
                    if act_cast:
                        # tail pairs: cast on the (by then idle) scalar
                        # engine so the pv psum slots don't wait behind the
                        # final muls in the DVE stream
                        nc.scalar.copy(ob, pv)
                    else:
                        nc.vector.tensor_copy(ob, pv)
                    nc.sync.dma_start(
                        out=outT[h, :, qh * 512:(qh + 1) * 512], in_=ob)

            # ---- schedule: emission order == per-engine execution order, so
            # it is arranged to match DMA arrival; PV lags its pair's scores
            # by one pair so the in-order PE stream never blocks on a late E
            # arrival ----
            pend = []

            def pair(hp, qh, act_cast=False):
                ets = emit_scores(hp, qh)
                if pend:
                    emit_pv(pend.pop(), act_cast)
                pend.append((hp, qh, ets))

            emit_proj_q(0, 0)
            emit_proj_k(0)
            pair(0, 0)
            emit_proj_v()
            emit_proj_q(0, 1)
            pair(0, 1)
            emit_proj_q(1, 0)
            emit_proj_q(1, 1)
            emit_proj_k(1)
            pair(1, 0)
            pair(1, 1)
            emit_proj_q(2, 0)
            emit_proj_q(2, 1)
            emit_proj_k(2)
            pair(2, 0)
            pair(2, 1)
            emit_pv(pend.pop())

            for cm in reversed(_psum_cms):
                cm.__exit__(None, None, None)

    nc.compile()
    return nc


def _get_compiled(kpad):
    if kpad not in _compiled:
        _compiled[kpad] = _build(kpad)
    return _compiled[kpad]


def kernel(hidden_states, Wq, bq, Wk, bk, Wv, bv, rel_pos, rel_2d_pos,
           attention_mask, _trace=False):
    global last_result

    hidden_states = np.asarray(hidden_states, np.float32)
    Wq, Wk, Wv = (np.asarray(w, np.float32) for w in (Wq, Wk, Wv))
    bq, bk, bv = (np.asarray(x, np.float32) for x in (bq, bk, bv))
    rel_pos = np.asarray(rel_pos, np.float32)
    rel_2d_pos = np.asarray(rel_2d_pos, np.float32)
    attention_mask = np.asarray(attention_mask, np.int32)

    # compact k to unmasked key positions (masked keys get probability 0)
    keeps = [np.where(attention_mask[b, 0, 0] == 0)[0] for b in range(B)]
    max_kc = max(len(k) for k in keeps)
    kpad = max(128, -(-max_kc // 64) * 64)
    n_full, rem = kpad // 128, kpad % 128
    nc = _get_compiled(kpad)

    in_maps = []
    for c in range(N_CORES):
        b, hg = divmod(c, 2)
        cs = slice(hg * COLS, (hg + 1) * COLS)
        h0 = hg * HPC
        keep = keeps[b]
        k_c = len(keep)
        hkT = np.zeros((H, kpad), BF16_NP)
        hkT[:, :k_c] = hidden_states[b][keep].T
        # E = exp(rel1+rel2) compacted along k and transposed to [k, q];
        # zero at padding -> those keys get weight exactly 0.
        r = (rel_pos[b, h0:h0 + HPC][:, :, keep]
             + rel_2d_pos[b, h0:h0 + HPC][:, :, keep])
        eh = np.zeros((HPC, kpad, S), BF16_NP)
        eh[:, :k_c, :] = np.exp(r).transpose(0, 2, 1)

        def onchip(a, nchunk):
            # [c*128+p, n] -> [p, c, n] (pre-applied DMA rearrange)
            return np.ascontiguousarray(
                a.reshape(nchunk, 128, a.shape[-1]).transpose(1, 0, 2))

        def onchip_hp(a):
            # [c*128+p, hp*128+m] -> [p, hp, c, m]
            return np.ascontiguousarray(
                a.reshape(KC, 128, NHP, 128).transpose(1, 2, 0, 3))

        im = {
            # [p, sh, c, m] layout (sh-major so each half unblocks early)
            "hsT": np.ascontiguousarray(
                hidden_states[b].T.astype(BF16_NP)
                .reshape(KC, 128, QH, 512).transpose(1, 2, 0, 3)),
            "hkT": onchip(hkT, KC),
            "wq": onchip_hp((Wq[:, cs] * np.float32(0.125)).astype(BF16_NP)),
            "wk": onchip_hp(Wk[:, cs].astype(BF16_NP)),
            "wv": onchip(Wv[:, cs].astype(BF16_NP), KC),
            "bq": np.ascontiguousarray(bq[cs]) * np.float32(0.125),
            "bk": np.ascontiguousarray(bk[cs]),
            "bv": np.ascontiguousarray(bv[cs]),
            # [h, p, c, q] for the full 128-chunks; remainder separate
            "ehA": np.ascontiguousarray(
                eh[:, :n_full * 128, :].reshape(HPC, n_full, 128, S)
                .transpose(0, 2, 1, 3)),
        }
        if rem:
            im["ehB"] = np.ascontiguousarray(eh[:, n_full * 128:, :])
        in_maps.append(im)

    kwargs = {}
    if _trace or os.environ.get("KERNEL_TRACE"):
        kwargs["trace"] = True
    last_result = run_bass_kernel_spmd(nc, in_maps, list(range(N_CORES)), **kwargs)

    result = np.empty((B, S, H), np.float32)
    for c in range(N_CORES):
        b, hg = divmod(c, 2)
        ot = np.asarray(last_result.results[c]["outT"], np.float32)
        ctx = ot[:, 0:HD, :] / ot[:, HD:HD + 1, :]       # [HPC, HD, S]
        result[b, :, hg * COLS:(hg + 1) * COLS] = (
            ctx.transpose(2, 0, 1).reshape(S, COLS))
    return result
